# revision 10
# baseline (speedup 1.0000x reference)
"""Trainium2 Bass kernel for the entropy-regularized knapsack CVX loss.

Math: with e = x / (||x||_2 * TAU), the per-row solution of
    max e@z + EPS*sum(entr(z))  s.t. 0<=z<=1, sum z = K
is p_i = min(1, exp((e_i - nu)/EPS - 1)) with nu s.t. sum_i p_i = K.
Since |e_i| <= 1 (Cauchy-Schwarz) and n = 8192 >> K*e^2, the min(1,.)
clamp can never be active at the optimum (sum exp(e_i) >= n/e > K*e), so
nu has the closed form nu + 1 = log(sum_i exp(e_i) / K), i.e.
p = K * softmax(e).  The loss is mean(-log(p_y + 1e-8)).

Device work per 128-row tile (data-parallel over 8 cores, 8 tiles/core):
  DVE : ss = sum(x^2)            (tensor_tensor_reduce, one pass)
  ACT : invnorm = exp(-0.5*ln(ss))   (Ln+Exp share one ACT table set)
  ACT : s = sum(exp(x * invnorm))    (activation w/ per-partition scale,
                                      fused accumulate, one pass)
Host: gather x[r, y[r]], nu = log(s/K)-1, loss mean; exact f64 fallback
for any row that fails the finiteness/no-clip guard (never for real data).
"""

import numpy as np

_BATCH = 8192
_N = 8192
_NCORES = 8
_RPC = _BATCH // _NCORES  # rows per core
_P = 128
_TILES = _RPC // _P  # row-tiles per core
_K = 5.0
_TAU = 1.0
_EPS = 1.0

_NC_CACHE = {}


def _build_bass(repeat=1):
    import concourse.bacc as bacc
    import concourse.mybir as mybir
    import concourse.tile as tile

    nc = bacc.Bacc("TRN2", target_bir_lowering=False, debug=False, num_devices=_NCORES)
    x = nc.dram_tensor("x", [_RPC, _N], mybir.dt.float32, kind="ExternalInput")
    stats = nc.dram_tensor(
        "stats", [_P, 2 * _TILES], mybir.dt.float32, kind="ExternalOutput"
    )

    f32 = mybir.dt.float32
    AF = mybir.ActivationFunctionType
    with tile.TileContext(nc) as tc:
        with (
            tc.tile_pool(name="xp", bufs=3) as xp,
            tc.tile_pool(name="sp", bufs=2) as sp,
            tc.tile_pool(name="smalls", bufs=3) as smalls,
            tc.tile_pool(name="singles", bufs=1) as singles,
        ):
            stats_sb = singles.tile([_P, 2 * _TILES], f32)

            def body():
              for t in range(_TILES):
                x_tile = xp.tile([_P, _N], f32, tag="x", name=f"x_{t}")
                nc.sync.dma_start(out=x_tile, in_=x[t * _P : (t + 1) * _P, :])

                sq = sp.tile([_P, _N], f32, tag="scratch", name=f"sq_{t}")
                ss = smalls.tile([_P, 1], f32, tag="ss", name=f"ss_{t}")
                nc.scalar.activation(sq, x_tile, AF.Square, accum_out=ss)
                lns = smalls.tile([_P, 1], f32, tag="lns", name=f"lns_{t}")
                nc.scalar.activation(lns, ss, AF.Ln)
                inv = stats_sb[:, _TILES + t : _TILES + t + 1]
                nc.scalar.activation(inv, lns, AF.Exp, scale=-0.5)

                ex = sp.tile([_P, _N], f32, tag="scratch", name=f"ex_{t}")
                nc.scalar.activation(
                    ex,
                    x_tile,
                    AF.Exp,
                    scale=inv,
                    accum_out=stats_sb[:, t : t + 1],
                )

            if repeat == 1:
                body()
            else:
                with tc.For_i(0, repeat, 1):
                    body()
            nc.sync.dma_start(out=stats[:, :], in_=stats_sb)
    nc.finalize()
    return nc


def _get_nc(repeat=1):
    if repeat not in _NC_CACHE:
        _NC_CACHE[repeat] = _build_bass(repeat)
    return _NC_CACHE[repeat]


def _exact_p_y(xrows, yrows):
    """f64 exact solve of the knapsack dual for fallback rows."""
    xr = np.asarray(xrows, dtype=np.float64)
    n = xr.shape[1]
    norm = np.maximum(np.sqrt((xr * xr).sum(1, keepdims=True)), 1e-12)
    e = xr / norm / _TAU
    lo = e.min(1) - _EPS
    hi = e.max(1) + _EPS * np.log(float(n))
    for _ in range(200):
        mid = 0.5 * (lo + hi)
        f = np.minimum(1.0, np.exp((e - mid[:, None]) / _EPS - 1.0)).sum(1)
        big = f > _K
        lo = np.where(big, mid, lo)
        hi = np.where(big, hi, mid)
    nu = 0.5 * (lo + hi)
    e_y = e[np.arange(e.shape[0]), yrows]
    return np.minimum(1.0, np.exp((e_y - nu) / _EPS - 1.0))


def kernel(x, y):
    from concourse.bass_utils import run_bass_kernel_spmd

    x = np.asarray(x, dtype=np.float32)
    y = np.asarray(y).astype(np.int64)
    assert x.shape == (_BATCH, _N)

    nc = _get_nc()
    in_maps = [
        {"x": np.ascontiguousarray(x[i * _RPC : (i + 1) * _RPC])}
        for i in range(_NCORES)
    ]
    res = run_bass_kernel_spmd(nc, in_maps, core_ids=list(range(_NCORES)))

    s_parts = []
    inv_parts = []
    for r in res.results:
        st = r["stats"]
        s_parts.append(st[:, :_TILES].T.reshape(-1))
        inv_parts.append(st[:, _TILES:].T.reshape(-1))
    s = np.concatenate(s_parts).astype(np.float64)
    invnorm = np.concatenate(inv_parts).astype(np.float64)

    rows = np.arange(_BATCH)
    x_y = x[rows, y].astype(np.float64)
    e_y = x_y * invnorm / _TAU
    with np.errstate(all="ignore"):
        nu1 = np.log(s / _K)  # nu + 1
        p_y = np.minimum(1.0, np.exp(e_y - nu1))
        # no-clip guard: impossible for finite inputs of this shape, but
        # catches NaN/Inf propagation (e.g. an all-zero row).
        bad = ~(np.isfinite(p_y) & (s > _K * np.e))
    if bad.any():
        p_y[bad] = _exact_p_y(x[bad], y[bad])
    loss = np.mean(-np.log(p_y + 1e-8))
    return np.array(loss, dtype=np.float32)


# revision 23
# speedup vs baseline: 1.8798x; 1.8798x over previous
"""Trainium2 Bass kernel for the entropy-regularized knapsack CVX loss.

Math: with e = x / (||x||_2 * TAU), the per-row solution of
    max e@z + EPS*sum(entr(z))  s.t. 0<=z<=1, sum z = K
is p_i = min(1, exp((e_i - nu)/EPS - 1)) with nu s.t. sum_i p_i = K.
Since |e_i| <= 1 (Cauchy-Schwarz) and n = 8192 >> K*e^2, the min(1,.)
clamp can never be active at the optimum (sum exp(e_i) >= n/e > K*e), so
nu has the closed form nu + 1 = log(sum_i exp(e_i) / K), i.e.
p = K * softmax(e).  The loss is mean(-log(p_y + 1e-8)).

Device work per 128-row tile (data-parallel over 8 cores, 8 tiles/core):
  ss = sum(x^2)  -> invnorm = exp(-0.5*ln(ss))  (Ln/Exp/Square share one
  ACT table set), then s = sum(exp(x * invnorm)) via one ACT pass with
  per-partition scale and fused free-dim accumulation.  The ss reduction
  runs on ACT (Square+accum) for some tiles and on DVE (bn_stats) for
  others to balance engine occupancy against the DMA roofline.
Host: gather x[r, y[r]], nu = log(s/K)-1, loss mean; exact f64 fallback
for any row that fails the finiteness/no-clip guard (never for real data).
"""

import numpy as np

_BATCH = 8192
_N = 8192
_NCORES = 8
_RPC = _BATCH // _NCORES  # rows per core
_P = 128
_TILES = _RPC // _P  # row-tiles per core
_K = 5.0
_TAU = 1.0
_EPS = 1.0

_NC_CACHE = {}
VARIANT = "ttr16"  # default device variant used by kernel()


def _build_bass(repeat=1, variant="act"):
    import concourse.bacc as bacc
    import concourse.mybir as mybir
    import concourse.tile as tile

    nc = bacc.Bacc(
        "TRN2", target_bir_lowering=False, debug=False, num_devices=_NCORES
    )
    f32 = mybir.dt.float32
    bf16 = mybir.dt.bfloat16
    AF = mybir.ActivationFunctionType

    in16 = variant.endswith("16")
    if in16:
        variant = variant[:-2]
    x_dt = bf16 if in16 else f32

    x = nc.dram_tensor("x", [_RPC, _N], x_dt, kind="ExternalInput")
    stats = nc.dram_tensor(
        "stats", [_P, 2 * _TILES], mybir.dt.float32, kind="ExternalOutput"
    )

    # per-tile placement of the sum-of-squares reduction:
    # 'A' = ACT Square+accum, 'B' = DVE bn_stats, 'T' = DVE custom
    # TENSOR_TENSOR_REDUCE (one instruction)
    if variant == "act":
        modes = "A" * _TILES
    elif variant == "bn":
        modes = "B" * _TILES
    elif variant == "mix":
        modes = "ABBABBAB"
    elif variant == "mix1":
        modes = "BBBBABBB"
    elif variant == "mix2":
        modes = "BBABBABB"
    elif variant == "ttr":
        modes = "T" * _TILES
    elif variant == "ttrmix1":
        modes = "TTTTATTT"
    elif variant == "ttrp":
        modes = "T" * _TILES
    elif variant == "dma":
        modes = "A" * _TILES
    else:
        raise ValueError(variant)
    psum_exp = variant == "ttrp"

    with tile.TileContext(nc) as tc:
        with (
            tc.tile_pool(name="xp", bufs=6 if in16 else 4) as xp,
            tc.tile_pool(name="sp", bufs=3 if in16 else 2) as sp,
            tc.tile_pool(name="smalls", bufs=4) as smalls,
            tc.tile_pool(name="singles", bufs=1) as singles,
            tc.tile_pool(name="ps", bufs=1, space="PSUM") as ps,
        ):
            stats_sb = singles.tile([_P, 2 * _TILES], f32)
            if variant == "dma":
                nc.vector.memset(stats_sb, 0.0)

            def tile_body(t):
                x_tile = xp.tile([_P, _N], x_dt, tag="x", name=f"x_{t}")
                nc.sync.dma_start(out=x_tile, in_=x[t * _P : (t + 1) * _P, :])
                if variant == "dma":
                    return

                ss = smalls.tile([_P, 1], f32, tag="ss", name=f"ss_{t}")
                if modes[t] == "T":
                    from concourse.dve_ops import TENSOR_TENSOR_REDUCE

                    sq = sp.tile([_P, _N], bf16, tag="scratch", name=f"sq_{t}")
                    nc.vector._custom_dve(
                        TENSOR_TENSOR_REDUCE, out=sq, in0=x_tile, in1=x_tile,
                        s0=0.0, s1=1.0, imm2=0.0, accum_out=ss,
                    )
                    ln_scale = 1.0
                elif modes[t] == "B":
                    # DVE path: bn_stats gives mean/var per 512-chunk;
                    # ss = N * (var + mean^2)
                    g = _N // 512
                    xg = x_tile.rearrange("p (g d) -> p g d", d=512)
                    bnst = smalls.tile(
                        [_P, g, nc.vector.BN_STATS_DIM], f32, tag="bnst",
                        name=f"bnst_{t}",
                    )
                    for j in range(g):
                        nc.vector.bn_stats(out=bnst[:, j, :], in_=xg[:, j, :])
                    mv = smalls.tile(
                        [_P, nc.vector.BN_AGGR_DIM], f32, tag="mv",
                        name=f"mv_{t}",
                    )
                    nc.vector.bn_aggr(out=mv, in_=bnst)
                    sqm = smalls.tile([_P, 1], f32, tag="sqm", name=f"sqm_{t}")
                    nc.scalar.activation(sqm, mv[:, 0:1], AF.Square)
                    # ss/N = var + mean^2
                    nc.scalar.activation(
                        ss, sqm, AF.Identity, bias=mv[:, 1:2]
                    )
                    ln_scale = float(_N)
                else:
                    # ACT path: Square with fused accumulate
                    sq = sp.tile([_P, _N], bf16, tag="scratch", name=f"sq_{t}")
                    nc.scalar.activation(sq, x_tile, AF.Square, accum_out=ss)
                    ln_scale = 1.0

                lns = smalls.tile([_P, 1], f32, tag="lns", name=f"lns_{t}")
                nc.scalar.activation(lns, ss, AF.Ln, scale=ln_scale)
                inv = stats_sb[:, _TILES + t : _TILES + t + 1]
                nc.scalar.activation(inv, lns, AF.Exp, scale=-0.5)

                if psum_exp:
                    half = _N // 2
                    s0c = smalls.tile([_P, 1], f32, tag="s0c", name=f"s0c_{t}")
                    s1c = smalls.tile([_P, 1], f32, tag="s1c", name=f"s1c_{t}")
                    for h, sc in ((0, s0c), (1, s1c)):
                        exh = ps.tile([_P, half], f32, tag="ps", name=f"exh_{t}_{h}")
                        nc.scalar.activation(
                            exh, x_tile[:, h * half : (h + 1) * half],
                            AF.Exp, scale=inv, accum_out=sc,
                        )
                    nc.scalar.activation(
                        stats_sb[:, t : t + 1], s0c, AF.Identity, bias=s1c
                    )
                else:
                    ex = sp.tile([_P, _N], bf16, tag="scratch", name=f"ex_{t}")
                    nc.scalar.activation(
                        ex, x_tile, AF.Exp, scale=inv,
                        accum_out=stats_sb[:, t : t + 1],
                    )

            def body():
                for t in range(_TILES):
                    tile_body(t)

            if repeat == 1:
                body()
            else:
                with tc.For_i(0, repeat, 1):
                    body()
            nc.sync.dma_start(out=stats[:, :], in_=stats_sb)
    nc.finalize()
    return nc


def _get_nc(repeat=1, variant=None):
    if variant is None:
        variant = VARIANT
    key = (repeat, variant)
    if key not in _NC_CACHE:
        _NC_CACHE[key] = _build_bass(repeat, variant)
    return _NC_CACHE[key]


def _exact_p_y(xrows, yrows):
    """f64 exact solve of the knapsack dual for fallback rows."""
    xr = np.asarray(xrows, dtype=np.float64)
    n = xr.shape[1]
    norm = np.maximum(np.sqrt((xr * xr).sum(1, keepdims=True)), 1e-12)
    e = xr / norm / _TAU
    lo = e.min(1) - _EPS
    hi = e.max(1) + _EPS * np.log(float(n))
    for _ in range(200):
        mid = 0.5 * (lo + hi)
        f = np.minimum(1.0, np.exp((e - mid[:, None]) / _EPS - 1.0)).sum(1)
        big = f > _K
        lo = np.where(big, mid, lo)
        hi = np.where(big, hi, mid)
    nu = 0.5 * (lo + hi)
    e_y = e[np.arange(e.shape[0]), yrows]
    return np.minimum(1.0, np.exp((e_y - nu) / _EPS - 1.0))


def kernel(x, y):
    from concourse.bass_utils import run_bass_kernel_spmd

    x = np.asarray(x, dtype=np.float32)
    y = np.asarray(y).astype(np.int64)
    assert x.shape == (_BATCH, _N)

    nc = _get_nc()
    if VARIANT.endswith("16"):
        import ml_dtypes

        xs = x.astype(ml_dtypes.bfloat16)
    else:
        xs = x
    in_maps = [
        {"x": np.ascontiguousarray(xs[i * _RPC : (i + 1) * _RPC])}
        for i in range(_NCORES)
    ]
    res = run_bass_kernel_spmd(nc, in_maps, core_ids=list(range(_NCORES)))

    s_parts = []
    inv_parts = []
    for r in res.results:
        st = r["stats"]
        s_parts.append(st[:, :_TILES].T.reshape(-1))
        inv_parts.append(st[:, _TILES:].T.reshape(-1))
    s = np.concatenate(s_parts).astype(np.float64)
    invnorm = np.concatenate(inv_parts).astype(np.float64)

    rows = np.arange(_BATCH)
    x_y = x[rows, y].astype(np.float64)
    e_y = x_y * invnorm / _TAU
    with np.errstate(all="ignore"):
        nu1 = np.log(s / _K)  # nu + 1
        p_y = np.minimum(1.0, np.exp(e_y - nu1))
        # no-clip guard: impossible for finite inputs of this shape, but
        # catches NaN/Inf propagation (e.g. an all-zero row).
        bad = ~(np.isfinite(p_y) & (s > _K * np.e))
    if bad.any():
        p_y[bad] = _exact_p_y(x[bad], y[bad])
    loss = np.mean(-np.log(p_y + 1e-8))
    return np.array(loss, dtype=np.float32)
